# revision 27
# baseline (speedup 1.0000x reference)
"""Windowed spatial MSA with precomputed attention — TRN2 Bass kernel.

Math: per window n (64 tokens, C=256):
    out_n = attn_n @ (x_n @ Wv) @ Wo + bo
         == attn_n @ x_n @ (Wv @ Wo) + bo          (associativity)

Host folds W = Wv @ Wo (256x256) and pre-permutes all operands into the
exact SBUF layouts, so every DMA is a flat contiguous 128-partition copy
and the device needs no on-chip transposes:
  mm1: T^T = lhsT(X_w).T @ rhs(A_w^T)   -> [C, tok] in PSUM  (per window)
  mm2: out = lhsT(T^T).T @ rhs(W)       -> [tok, C] accumulated over C chunks

All matmul operands are bf16 (fp32 PSUM accumulation): 1 cycle/row on the
PE vs 4 for fp32, and half the DMA bytes. Output is written bf16 and
upcast on host (rel err ~4e-3, well under the 2e-2 gate).

Sharding: data-parallel over the 128 window-row-groups (b, i): 16 per core,
processed as 8 pairs; each pair packs two groups on the 128 SBUF partitions.
"""

from collections import deque

import numpy as np
import ml_dtypes

import concourse.bacc as bacc
import concourse.mybir as mybir
from concourse.tile import TileContext
from concourse.bass_utils import run_bass_kernel_spmd

N_CORES = 8
B, H, WD, C = 4, 256, 256, 256
WS = 8
SEQ = WS * WS  # 64
NWJ = WD // WS  # 32 windows per row-group
GROUPS = B * (H // WS)  # 128 row-groups of 8 pixel rows
G_PER_CORE = GROUPS // N_CORES  # 16
GPAIRS = G_PER_CORE // 2  # 8
CH = C // 128  # 2 contraction chunks of 128

F32 = mybir.dt.float32
BF16 = mybir.dt.bfloat16


def build_nc(
    dt_mm=BF16,
    dt_out=BF16,
    gpairs=GPAIRS,
    nwj=NWJ,
    num_devices=N_CORES,
    x_split=4,
    a_split=2,
    x_bufs=3,
    o_split=2,
    o_bufs=2,
    reps=1,
    shared_t=True,
    defer=True,
):
    """Build the per-core Bass module (SPMD: all cores run the same program).

    DRAM layouts (host pre-permuted):
      x:   [gpairs, 128, nwj*C]   partitions=(g2,p,q) tokens, free=(j,c)
      at:  [gpairs, 128, nwj*SEQ] partitions=(g2,k),         free=(j,m)
      w:   [128, CH*C]            partitions=c_lo,           free=(ci,n)
      out: [gpairs, 128, nwj*C]   same layout as x; host un-permutes
    """
    NWJ = nwj
    GPAIRS = gpairs
    nc = bacc.Bacc(
        "TRN2", target_bir_lowering=False, debug=False, num_devices=num_devices
    )
    x = nc.dram_tensor("x", [GPAIRS, 128, NWJ * C], dt_mm, kind="ExternalInput")
    at = nc.dram_tensor("at", [GPAIRS, 128, NWJ * SEQ], dt_mm, kind="ExternalInput")
    w = nc.dram_tensor("w", [128, CH * C], dt_mm, kind="ExternalInput")
    out = nc.dram_tensor("out", [GPAIRS, 128, NWJ * C], dt_out, kind="ExternalOutput")

    with TileContext(nc) as tc:
        with (
            tc.tile_pool(name="wpool", bufs=1) as wpool,
            tc.tile_pool(name="xpool", bufs=x_bufs) as xpool,
            tc.tile_pool(name="apool", bufs=2) as apool,
            tc.tile_pool(name="tpool", bufs=4) as tpool,
            tc.tile_pool(name="obpool", bufs=o_bufs) as obpool,
            tc.tile_pool(name="tpsum", bufs=6, space="PSUM") as tpsum,
            tc.tile_pool(name="opsum", bufs=2, space="PSUM") as opsum,
        ):
            w_sb = wpool.tile([128, CH * C], dt_mm)
            nc.sync.dma_start(w_sb[:], w[:])

            import contextlib

            rep_ctx = (
                tc.For_i(
                    0,
                    reps,
                    1,
                    hint_engines=(
                        mybir.EngineType.PE,
                        mybir.EngineType.DVE,
                        mybir.EngineType.SP,
                        mybir.EngineType.Activation,
                    ),
                )
                if reps > 1
                else contextlib.nullcontext()
            )
            with rep_ctx:
                for gp in range(GPAIRS):
                    x_sb = xpool.tile([128, NWJ * C], dt_mm, tag="x", name="x_sb")
                    xw = NWJ * C // x_split
                    for s in range(x_split):
                        nc.sync.dma_start(
                            x_sb[:, s * xw : (s + 1) * xw], x[gp, :, s * xw : (s + 1) * xw]
                        )
                    a_sb = apool.tile([128, NWJ * SEQ], dt_mm, tag="a", name="a_sb")
                    aw = NWJ * SEQ // a_split
                    for s in range(a_split):
                        nc.sync.dma_start(
                            a_sb[:, s * aw : (s + 1) * aw], at[gp, :, s * aw : (s + 1) * aw]
                        )

                    ob = obpool.tile([128, NWJ * C], dt_out, tag="ob", name="ob")

                    # mm2 for window j is deferred to iteration j+1 so the
                    # PSUM->SBUF copy of T^T has a full mm1 block to complete
                    # under, keeping the PE stall-free.
                    def emit_mm2(j, t_sb, ob=ob):
                        o_ps = opsum.tile([128, C], F32, tag="ops", name="o_ps")
                        for ci in range(CH):
                            nc.tensor.matmul(
                                o_ps[:],
                                lhsT=t_sb[:, ci * 128 : (ci + 1) * 128],
                                rhs=w_sb[:, ci * C : (ci + 1) * C],
                                start=(ci == 0),
                                stop=(ci == CH - 1),
                            )
                        if j % 2 == 1:
                            nc.scalar.copy(ob[:, j * C : (j + 1) * C], o_ps[:])
                        else:
                            nc.vector.tensor_copy(ob[:, j * C : (j + 1) * C], o_ps[:])

                    prev = None
                    for j in range(NWJ):
                        # mm1 per window half and C-chunk: T^T[c,tok] = X_w.T @ A_w^T.
                        # All four products cover the full 128 partitions, so they
                        # can share one PSUM tile (disjoint column ranges) and be
                        # evicted with a single copy. Columns: (c0h0,c0h1,c1h0,c1h1)
                        # = exactly the [c,m] layout mm2's lhsT slices need.
                        t_sb = tpool.tile([128, CH * 128], dt_mm, tag="tsb", name="t_sb")
                        if shared_t:
                            t_ps = tpsum.tile([128, CH * 128], F32, tag="tps", name="t_ps")
                            for ci in range(CH):
                                for half, base in ((0, 0), (1, 64)):
                                    col = ci * 128 + half * 64
                                    nc.tensor.matmul(
                                        t_ps[:, col : col + 64],
                                        lhsT=x_sb[
                                            base : base + 64,
                                            j * C + ci * 128 : j * C + ci * 128 + 128,
                                        ],
                                        rhs=a_sb[base : base + 64, j * SEQ : (j + 1) * SEQ],
                                        start=True,
                                        stop=True,
                                    )
                            if j % 2 == 0:
                                nc.vector.tensor_copy(t_sb[:], t_ps[:])
                            else:
                                nc.scalar.copy(t_sb[:], t_ps[:])
                        else:
                            for ci in range(CH):
                                t_ps = [
                                    tpsum.tile([128, 64], F32, tag="tps", name=f"tps{ci}{h}")
                                    for h in range(2)
                                ]
                                for half, base in ((0, 0), (1, 64)):
                                    nc.tensor.matmul(
                                        t_ps[half][:],
                                        lhsT=x_sb[
                                            base : base + 64,
                                            j * C + ci * 128 : j * C + ci * 128 + 128,
                                        ],
                                        rhs=a_sb[base : base + 64, j * SEQ : (j + 1) * SEQ],
                                        start=True,
                                        stop=True,
                                    )
                                nc.vector.tensor_copy(
                                    t_sb[:, ci * 128 : ci * 128 + 64], t_ps[0][:]
                                )
                                nc.scalar.copy(
                                    t_sb[:, ci * 128 + 64 : ci * 128 + 128], t_ps[1][:]
                                )
                        if defer and prev is not None:
                            emit_mm2(prev[0], prev[1])
                        elif not defer:
                            emit_mm2(j, t_sb)
                        prev = (j, t_sb)
                    if defer:
                        emit_mm2(prev[0], prev[1])
                    ow = NWJ * C // o_split
                    for s in range(o_split):
                        nc.sync.dma_start(
                            out[gp, :, s * ow : (s + 1) * ow], ob[:, s * ow : (s + 1) * ow]
                        )
    nc.compile()
    return nc


def build_nc_axp(
    dt_mm=BF16,
    dt_out=BF16,
    gpairs=GPAIRS,
    nwj=NWJ,
    num_devices=N_CORES,
    x_split=4,
    a_split=2,
    x_bufs=3,
    o_split=2,
    o_bufs=2,
    t_bufs=3,
    reps=1,
):
    """A@X-first with window-PAIRED PSUM tiles.

    Same dataflow and host layouts as build_nc, but each PSUM tile is
    [128,512] fp32 = exactly one bank covering TWO windows, written by a
    single accumulation group (one has_written clear; disjoint column
    ranges overwrite-where-unset). Eviction is ONE [128,512] V/S copy per
    two windows, so the copy fixed overhead no longer dominates.
    """
    NWJ = nwj
    GPAIRS = gpairs
    assert NWJ % 2 == 0
    nc = bacc.Bacc(
        "TRN2", target_bir_lowering=False, debug=False, num_devices=num_devices
    )
    x = nc.dram_tensor("x", [GPAIRS, 128, NWJ * C], dt_mm, kind="ExternalInput")
    at = nc.dram_tensor("at", [GPAIRS, 128, NWJ * SEQ], dt_mm, kind="ExternalInput")
    w = nc.dram_tensor("w", [128, CH * C], dt_mm, kind="ExternalInput")
    out = nc.dram_tensor("out", [GPAIRS, 128, NWJ * C], dt_out, kind="ExternalOutput")

    with TileContext(nc) as tc:
        with (
            tc.tile_pool(name="wpool", bufs=1) as wpool,
            tc.tile_pool(name="xpool", bufs=x_bufs) as xpool,
            tc.tile_pool(name="apool", bufs=2) as apool,
            tc.tile_pool(name="tpool", bufs=t_bufs) as tpool,
            tc.tile_pool(name="obpool", bufs=o_bufs) as obpool,
            tc.tile_pool(name="tpsum", bufs=2, space="PSUM") as tpsum,
            tc.tile_pool(name="opsum", bufs=2, space="PSUM") as opsum,
        ):
            w_sb = wpool.tile([128, CH * C], dt_mm)
            nc.sync.dma_start(w_sb[:], w[:])

            import contextlib

            rep_ctx = (
                tc.For_i(
                    0,
                    reps,
                    1,
                    hint_engines=(
                        mybir.EngineType.PE,
                        mybir.EngineType.DVE,
                        mybir.EngineType.SP,
                        mybir.EngineType.Activation,
                    ),
                )
                if reps > 1
                else contextlib.nullcontext()
            )
            with rep_ctx:
                for gp in range(GPAIRS):
                    x_sb = xpool.tile([128, NWJ * C], dt_mm, tag="x", name="x_sb")
                    xw = NWJ * C // x_split
                    for s in range(x_split):
                        nc.sync.dma_start(
                            x_sb[:, s * xw : (s + 1) * xw], x[gp, :, s * xw : (s + 1) * xw]
                        )
                    a_sb = apool.tile([128, NWJ * SEQ], dt_mm, tag="a", name="a_sb")
                    aw = NWJ * SEQ // a_split
                    for s in range(a_split):
                        nc.sync.dma_start(
                            a_sb[:, s * aw : (s + 1) * aw], at[gp, :, s * aw : (s + 1) * aw]
                        )

                    ob = obpool.tile([128, NWJ * C], dt_out, tag="ob", name="ob")

                    def emit_mm2p(jp, t_sb, ob=ob):
                        o_ps = opsum.tile([128, 2 * C], F32, tag="ops", name="o_ps")
                        for u in range(2):
                            for ci in range(CH):
                                nc.tensor.matmul(
                                    o_ps[:, u * C : (u + 1) * C],
                                    lhsT=t_sb[:, u * C + ci * 128 : u * C + (ci + 1) * 128],
                                    rhs=w_sb[:, ci * C : (ci + 1) * C],
                                    start=(u == 0 and ci == 0),
                                    stop=(u == 1 and ci == CH - 1),
                                )
                        if jp % 2 == 0:
                            nc.scalar.copy(ob[:, 2 * jp * C : (2 * jp + 2) * C], o_ps[:])
                        else:
                            nc.vector.tensor_copy(
                                ob[:, 2 * jp * C : (2 * jp + 2) * C], o_ps[:]
                            )

                    prev = None
                    for jp in range(NWJ // 2):
                        # 8 mm1 matmuls (2 windows x 2 C-chunks x 2 halves) fill
                        # one full PSUM bank as a single accumulation group.
                        t_ps = tpsum.tile([128, 2 * C], F32, tag="tps", name="t_ps")
                        first = True
                        for u in range(2):
                            j = 2 * jp + u
                            for ci in range(CH):
                                for half, base in ((0, 0), (1, 64)):
                                    col = u * C + ci * 128 + half * 64
                                    nc.tensor.matmul(
                                        t_ps[:, col : col + 64],
                                        lhsT=x_sb[
                                            base : base + 64,
                                            j * C + ci * 128 : j * C + ci * 128 + 128,
                                        ],
                                        rhs=a_sb[
                                            base : base + 64, j * SEQ : (j + 1) * SEQ
                                        ],
                                        start=first,
                                        stop=(u == 1 and ci == CH - 1 and half == 1),
                                    )
                                    first = False
                        t_sb = tpool.tile([128, 2 * C], dt_mm, tag="tsb", name="t_sb")
                        if jp % 2 == 0:
                            nc.vector.tensor_copy(t_sb[:], t_ps[:])
                        else:
                            nc.scalar.copy(t_sb[:], t_ps[:])
                        if prev is not None:
                            emit_mm2p(prev[0], prev[1])
                        prev = (jp, t_sb)
                    emit_mm2p(prev[0], prev[1])
                    ow = NWJ * C // o_split
                    for s in range(o_split):
                        nc.sync.dma_start(
                            out[gp, :, s * ow : (s + 1) * ow], ob[:, s * ow : (s + 1) * ow]
                        )
    nc.compile()
    return nc


def build_nc_axp2(
    dt_mm=BF16,
    dt_out=BF16,
    gpairs=GPAIRS,
    nwj=NWJ,
    num_devices=N_CORES,
    x_split=1,
    a_split=1,
    x_bufs=3,
    o_split=1,
    o_bufs=2,
    t_bufs=4,
    jblk=4,
    defer_n=2,
    reps=1,
):
    """A@X-first, compact attn (36MB/core DMA), PSUM-bank-safe grouping.

    mm1 (T^T = X^T A^T) has K=64 contraction per window half; matmuls from
    different PE row-groups (lhsT partitions 0:64 vs 64:128) into one PSUM
    bank are a fatal HW collision, so each half gets its OWN bank: per
    jblk-window block, bank_h0 collects all row-group-0 products (one
    accumulation group of jblk*CH N=64 matmuls), bank_h1 likewise. The two
    evictions use strided destination APs to interleave halves into the
    [c, m-pair] layout mm2 needs. mm2 accumulates per window-pair into one
    bank (full-K groups), o_ps spans jblk/2 banks, one eviction per block.

    Host layouts identical to build_nc (make_in_maps).
    """
    NWJ = nwj
    GPAIRS = gpairs
    assert NWJ % jblk == 0 and jblk % 2 == 0
    nc = bacc.Bacc(
        "TRN2", target_bir_lowering=False, debug=False, num_devices=num_devices
    )
    x = nc.dram_tensor("x", [GPAIRS, 128, NWJ * C], dt_mm, kind="ExternalInput")
    at = nc.dram_tensor("at", [GPAIRS, 128, NWJ * SEQ], dt_mm, kind="ExternalInput")
    w = nc.dram_tensor("w", [128, CH * C], dt_mm, kind="ExternalInput")
    out = nc.dram_tensor("out", [GPAIRS, 128, NWJ * C], dt_out, kind="ExternalOutput")

    TW = jblk * CH * 64  # columns per half-bank tile (jblk windows x CH x 64)

    with TileContext(nc) as tc:
        with (
            tc.tile_pool(name="wpool", bufs=1) as wpool,
            tc.tile_pool(name="xpool", bufs=x_bufs) as xpool,
            tc.tile_pool(name="apool", bufs=2) as apool,
            tc.tile_pool(name="tpool", bufs=t_bufs) as tpool,
            tc.tile_pool(name="obpool", bufs=o_bufs) as obpool,
            tc.tile_pool(name="tpsum", bufs=2, space="PSUM") as tpsum,
            tc.tile_pool(name="opsum", bufs=2, space="PSUM") as opsum,
        ):
            w_sb = wpool.tile([128, CH * C], dt_mm)
            nc.sync.dma_start(w_sb[:], w[:])

            import contextlib

            rep_ctx = (
                tc.For_i(
                    0,
                    reps,
                    1,
                    hint_engines=(
                        mybir.EngineType.PE,
                        mybir.EngineType.DVE,
                        mybir.EngineType.SP,
                        mybir.EngineType.Activation,
                    ),
                )
                if reps > 1
                else contextlib.nullcontext()
            )
            with rep_ctx:
                for gp in range(GPAIRS):
                    x_sb = xpool.tile([128, NWJ * C], dt_mm, tag="x", name="x_sb")
                    xw = NWJ * C // x_split
                    for s in range(x_split):
                        nc.sync.dma_start(
                            x_sb[:, s * xw : (s + 1) * xw], x[gp, :, s * xw : (s + 1) * xw]
                        )
                    a_sb = apool.tile([128, NWJ * SEQ], dt_mm, tag="a", name="a_sb")
                    aw = NWJ * SEQ // a_split
                    for s in range(a_split):
                        nc.sync.dma_start(
                            a_sb[:, s * aw : (s + 1) * aw], at[gp, :, s * aw : (s + 1) * aw]
                        )

                    ob = obpool.tile([128, NWJ * C], dt_out, tag="ob", name="ob")

                    def emit_mm2(jq, t_sb, ob=ob):
                        o_ps = opsum.tile([128, jblk * C], F32, tag="ops", name="o_ps")
                        for v in range(jblk // 2):
                            for u in range(2):
                                b = 2 * v + u
                                for ci in range(CH):
                                    nc.tensor.matmul(
                                        o_ps[:, b * C : (b + 1) * C],
                                        lhsT=t_sb[
                                            :, (b * CH + ci) * 128 : (b * CH + ci + 1) * 128
                                        ],
                                        rhs=w_sb[:, ci * C : (ci + 1) * C],
                                        start=(u == 0 and ci == 0),
                                        stop=(u == 1 and ci == CH - 1),
                                    )
                        if jq % 2 == 0:
                            nc.scalar.copy(
                                ob[:, jblk * jq * C : jblk * (jq + 1) * C], o_ps[:]
                            )
                        else:
                            nc.vector.tensor_copy(
                                ob[:, jblk * jq * C : jblk * (jq + 1) * C], o_ps[:]
                            )

                    pend = deque()
                    for jq in range(NWJ // jblk):
                        # Per half: one bank, one accumulation group of
                        # jblk*CH K=64 matmuls (all same PE row-group).
                        t_ps = [
                            tpsum.tile([128, TW], F32, tag=f"tps{h}", name=f"t_ps{h}")
                            for h in range(2)
                        ]
                        for half, base in ((0, 0), (1, 64)):
                            first = True
                            for b in range(jblk):
                                j = jblk * jq + b
                                for ci in range(CH):
                                    col = (b * CH + ci) * 64
                                    nc.tensor.matmul(
                                        t_ps[half][:, col : col + 64],
                                        lhsT=x_sb[
                                            base : base + 64,
                                            j * C + ci * 128 : j * C + ci * 128 + 128,
                                        ],
                                        rhs=a_sb[
                                            base : base + 64, j * SEQ : (j + 1) * SEQ
                                        ],
                                        start=first,
                                        stop=(b == jblk - 1 and ci == CH - 1),
                                    )
                                    first = False
                        # Evict with interleaving: t_sb block (b,ci) holds
                        # [h0 64 | h1 64] columns = the m-pair order mm2 needs.
                        t_sb = tpool.tile([128, jblk * CH * 128], dt_mm, tag="tsb", name="t_sb")
                        tv = t_sb[:].rearrange("p (n c) -> p n c", n=jblk * CH)
                        for half in range(2):
                            sv = t_ps[half][:].rearrange("p (n c) -> p n c", n=jblk * CH)
                            dsl = tv[:, :, half * 64 : (half + 1) * 64]
                            if half == 0:
                                nc.vector.tensor_copy(dsl, sv)
                            else:
                                nc.scalar.copy(dsl, sv)
                        pend.append((jq, t_sb))
                        if len(pend) > defer_n:
                            emit_mm2(*pend.popleft())
                    while pend:
                        emit_mm2(*pend.popleft())
                    ow = NWJ * C // o_split
                    for s in range(o_split):
                        nc.gpsimd.dma_start(
                            out[gp, :, s * ow : (s + 1) * ow], ob[:, s * ow : (s + 1) * ow]
                        )
    nc.compile()
    return nc


def build_nc_yf(
    dt_mm=BF16,
    dt_out=BF16,
    gpairs=GPAIRS,
    nwj=NWJ,
    num_devices=N_CORES,
    x_split=1,
    a_split=1,
    x_bufs=3,
    o_split=1,
    o_bufs=2,
    y_bufs=4,
    jblk=4,
    defer_n=2,
    reps=1,
):
    """Y-first association: Y = X @ W per window-pair, then out = A @ Y.

    Per window-pair j (2 windows, 128 tokens on partitions):
      stage1: Y[tok,n] += lhsT(X^T chunk)[c,tok].T @ rhs(W chunk)[c,n]  (2 C-chunks,
              one accumulation group -> one PSUM bank)
      stage2: out[m,n] = lhsT(blkdiag A^T)[k,m].T @ rhs(Y_sb)[k,n]      (1 matmul)

    PSUM tiles pair two consecutive windows ([128,512] = 2 banks, one
    accumulation group per bank), so eviction is ONE [128,512] copy per 2
    windows on V/S — the copy fixed-overhead no longer dominates.

    DRAM layouts (host pre-permuted):
      x:   [gpairs, 128, nwj*CH*128]  partitions=c_lo, free=(j,ci,(g2 p q))
      at:  [gpairs, 128, nwj*128]     partitions=(g2,k), free=(j,(g2',m)) blkdiag
      w:   [128, CH*C]                partitions=c_lo, free=(ci,n)
      out: [gpairs, 128, nwj*C]       partitions=(g2 p q), free=(j,n)
    """
    NWJ = nwj
    GPAIRS = gpairs
    assert NWJ % jblk == 0
    nc = bacc.Bacc(
        "TRN2", target_bir_lowering=False, debug=False, num_devices=num_devices
    )
    x = nc.dram_tensor("x", [GPAIRS, 128, NWJ * CH * 128], dt_mm, kind="ExternalInput")
    at = nc.dram_tensor("at", [GPAIRS, 128, NWJ * 128], dt_mm, kind="ExternalInput")
    w = nc.dram_tensor("w", [128, CH * C], dt_mm, kind="ExternalInput")
    out = nc.dram_tensor("out", [GPAIRS, 128, NWJ * C], dt_out, kind="ExternalOutput")

    with TileContext(nc) as tc:
        with (
            tc.tile_pool(name="wpool", bufs=1) as wpool,
            tc.tile_pool(name="xpool", bufs=x_bufs) as xpool,
            tc.tile_pool(name="apool", bufs=2) as apool,
            tc.tile_pool(name="ypool", bufs=y_bufs) as ypool,
            tc.tile_pool(name="obpool", bufs=o_bufs) as obpool,
            tc.tile_pool(name="ypsum", bufs=2, space="PSUM") as ypsum,
            tc.tile_pool(name="opsum", bufs=2, space="PSUM") as opsum,
        ):
            w_sb = wpool.tile([128, CH * C], dt_mm)
            nc.sync.dma_start(w_sb[:], w[:])

            import contextlib

            rep_ctx = (
                tc.For_i(
                    0,
                    reps,
                    1,
                    hint_engines=(
                        mybir.EngineType.PE,
                        mybir.EngineType.DVE,
                        mybir.EngineType.SP,
                        mybir.EngineType.Activation,
                    ),
                )
                if reps > 1
                else contextlib.nullcontext()
            )
            with rep_ctx:
                for gp in range(GPAIRS):
                    x_sb = xpool.tile([128, NWJ * CH * 128], dt_mm, tag="x", name="x_sb")
                    xw = NWJ * CH * 128 // x_split
                    for s in range(x_split):
                        nc.sync.dma_start(
                            x_sb[:, s * xw : (s + 1) * xw], x[gp, :, s * xw : (s + 1) * xw]
                        )
                    a_sb = apool.tile([128, NWJ * 128], dt_mm, tag="a", name="a_sb")
                    aw = NWJ * 128 // a_split
                    for s in range(a_split):
                        nc.sync.dma_start(
                            a_sb[:, s * aw : (s + 1) * aw], at[gp, :, s * aw : (s + 1) * aw]
                        )

                    ob = obpool.tile([128, NWJ * C], dt_out, tag="ob", name="ob")

                    # jblk windows per PSUM tile: [128, jblk*C] fp32 spans
                    # jblk/2 banks; each 2KB bank holds exactly ONE
                    # accumulation group (one window-pair: one has_written
                    # clear at start, disjoint column ranges overwrite-where-
                    # unset). Two groups in one bank is a fatal PE/PSUM
                    # collision on HW. Big tiles amortize the 120-170-cycle
                    # fixed cost of each V/S eviction copy; stage2 is deferred
                    # defer_n block-iterations so Y evictions hide under later
                    # stage1 work instead of stalling the PE.
                    def emit_s2(jq, y_sb, ob=ob):
                        o_ps = opsum.tile([128, jblk * C], F32, tag="ops", name="o_ps")
                        for v in range(jblk // 2):
                            for u in range(2):
                                b = 2 * v + u
                                j = jblk * jq + b
                                nc.tensor.matmul(
                                    o_ps[:, b * C : (b + 1) * C],
                                    lhsT=a_sb[:, j * 128 : (j + 1) * 128],
                                    rhs=y_sb[:, b * C : (b + 1) * C],
                                    start=(u == 0),
                                    stop=(u == 1),
                                )
                        if jq % 2 == 0:
                            nc.scalar.copy(
                                ob[:, jblk * jq * C : jblk * (jq + 1) * C], o_ps[:]
                            )
                        else:
                            nc.vector.tensor_copy(
                                ob[:, jblk * jq * C : jblk * (jq + 1) * C], o_ps[:]
                            )

                    pend = deque()
                    for jq in range(NWJ // jblk):
                        y_ps = ypsum.tile([128, jblk * C], F32, tag="yps", name="y_ps")
                        for v in range(jblk // 2):
                            for u in range(2):
                                b = 2 * v + u
                                j = jblk * jq + b
                                for ci in range(CH):
                                    nc.tensor.matmul(
                                        y_ps[:, b * C : (b + 1) * C],
                                        lhsT=x_sb[
                                            :, (j * CH + ci) * 128 : (j * CH + ci + 1) * 128
                                        ],
                                        rhs=w_sb[:, ci * C : (ci + 1) * C],
                                        start=(u == 0 and ci == 0),
                                        stop=(u == 1 and ci == CH - 1),
                                    )
                        y_sb = ypool.tile([128, jblk * C], dt_mm, tag="ysb", name="y_sb")
                        if jq % 2 == 0:
                            nc.vector.tensor_copy(y_sb[:], y_ps[:])
                        else:
                            nc.scalar.copy(y_sb[:], y_ps[:])
                        pend.append((jq, y_sb))
                        if len(pend) > defer_n:
                            emit_s2(*pend.popleft())
                    while pend:
                        emit_s2(*pend.popleft())
                    # Stores go on gpsimd (SWDGE): on sync's ring they'd wait
                    # for this gpair's compute and head-of-line block the next
                    # gpair's loads; on scalar's ring they'd contend with the
                    # eviction copies. gpsimd is otherwise idle.
                    ow = NWJ * C // o_split
                    for s in range(o_split):
                        nc.gpsimd.dma_start(
                            out[gp, :, s * ow : (s + 1) * ow], ob[:, s * ow : (s + 1) * ow]
                        )
    nc.compile()
    return nc


def make_in_maps_yf(x, attn, Wv, Wo, dt_mm=BF16):
    npdt = _np_dt(dt_mm)
    x = np.asarray(x, dtype=np.float32)
    attn = np.asarray(attn, dtype=np.float32)
    W = np.asarray(Wv, dtype=np.float32) @ np.asarray(Wo, dtype=np.float32)
    wprep = np.ascontiguousarray(
        W.reshape(CH, 128, C).transpose(1, 0, 2).reshape(128, CH * C).astype(npdt)
    )
    xg = x.astype(npdt).reshape(GROUPS, WS, NWJ, WS, C)
    ag = attn.astype(npdt).reshape(GROUPS, NWJ, SEQ, SEQ)  # [g, j, m, k]
    in_maps = []
    for cid in range(N_CORES):
        # x: [gp, c_lo, (j, ci, g2, p, q)]
        xs = xg[cid * G_PER_CORE : (cid + 1) * G_PER_CORE]
        xs = xs.reshape(GPAIRS, 2, WS, NWJ, WS, CH, 128)  # gp,g2,p,j,q,ci,c_lo
        xs = xs.transpose(0, 6, 3, 5, 1, 2, 4)  # gp,c_lo,j,ci,g2,p,q
        xs = np.ascontiguousarray(xs).reshape(GPAIRS, 128, NWJ * CH * 128)
        # at: block-diagonal [gp, (g2,k), (j, g2', m)]
        asl = ag[cid * G_PER_CORE : (cid + 1) * G_PER_CORE]
        asl = asl.reshape(GPAIRS, 2, NWJ, SEQ, SEQ)  # gp,g2,j,m,k
        abd = np.zeros((GPAIRS, 2, SEQ, NWJ, 2, SEQ), dtype=npdt)
        for g2 in range(2):
            # abd[gp, g2, k, j, g2, m] = A[gp, g2, j, m, k]
            abd[:, g2, :, :, g2, :] = asl[:, g2].transpose(0, 3, 1, 2)
        abd = abd.reshape(GPAIRS, 128, NWJ * 128)
        in_maps.append({"x": xs, "at": abd, "w": wprep})
    return in_maps


_NC_CACHE = {}


def get_nc(dt_mm=BF16, dt_out=BF16, reps=1, arch="yf", **kwargs):
    key = (arch, str(dt_mm), str(dt_out), reps, tuple(sorted(kwargs.items())))
    if key not in _NC_CACHE:
        fn = {"yf": build_nc_yf, "axp": build_nc_axp, "axp2": build_nc_axp2, "ax": build_nc}[arch]
        _NC_CACHE[key] = fn(dt_mm, dt_out, reps=reps, **kwargs)
    return _NC_CACHE[key]


def _np_dt(dt):
    return mybir.dt.np(dt)


def make_in_maps(x, attn, Wv, Wo, dt_mm=BF16):
    npdt = _np_dt(dt_mm)
    x = np.asarray(x, dtype=np.float32)
    attn = np.asarray(attn, dtype=np.float32)
    W = np.asarray(Wv, dtype=np.float32) @ np.asarray(Wo, dtype=np.float32)
    wprep = np.ascontiguousarray(
        W.reshape(CH, 128, C).transpose(1, 0, 2).reshape(128, CH * C).astype(npdt)
    )
    # x: (B,H,W,C) -> [g, p, j, q, c] -> per-core [gp, (g2 p q), (j c)]
    xg = x.astype(npdt).reshape(GROUPS, WS, NWJ, WS, C)
    ag = attn.astype(npdt).reshape(GROUPS, NWJ, SEQ, SEQ)  # [g, j, m, k]
    in_maps = []
    for cid in range(N_CORES):
        xs = xg[cid * G_PER_CORE : (cid + 1) * G_PER_CORE]
        xs = xs.reshape(GPAIRS, 2, WS, NWJ, WS, C).transpose(0, 1, 2, 4, 3, 5)
        xs = np.ascontiguousarray(xs).reshape(GPAIRS, 128, NWJ * C)
        asl = ag[cid * G_PER_CORE : (cid + 1) * G_PER_CORE]
        asl = asl.reshape(GPAIRS, 2, NWJ, SEQ, SEQ).transpose(0, 1, 4, 2, 3)
        asl = np.ascontiguousarray(asl).reshape(GPAIRS, 128, NWJ * SEQ)
        in_maps.append({"x": xs, "at": asl, "w": wprep})
    return in_maps


def assemble_out(results, bo):
    # out per core: [GPAIRS, 128=(g2 p q), NWJ*C] -> [G_PER_CORE, p, j, q, c]
    out = np.empty((GROUPS, WS, NWJ, WS, C), dtype=np.float32)
    for cid in range(N_CORES):
        r = results[cid]["out"].reshape(GPAIRS, 2, WS, WS, NWJ, C)
        r = r.transpose(0, 1, 2, 4, 3, 5).reshape(G_PER_CORE, WS, NWJ, WS, C)
        out[cid * G_PER_CORE : (cid + 1) * G_PER_CORE] = r.astype(np.float32)
    out = out.reshape(B, H, WD, C)
    bo = np.asarray(bo, dtype=np.float32)
    if np.any(bo):
        out = out + bo
    return out


def run(x, attn, Wv, Wo, bo, dt_mm=BF16, dt_out=BF16, reps=1, arch="yf", nc_kwargs=None, **spmd_kwargs):
    nc = get_nc(dt_mm, dt_out, reps=reps, arch=arch, **(nc_kwargs or {}))
    mk = make_in_maps_yf if arch == "yf" else make_in_maps
    in_maps = mk(x, attn, Wv, Wo, dt_mm=dt_mm)
    res = run_bass_kernel_spmd(nc, in_maps, core_ids=list(range(N_CORES)), **spmd_kwargs)
    return assemble_out(res.results, bo), res


def kernel(x, attn, Wv, Wo, bo):
    out, _ = run(x, attn, Wv, Wo, bo)
    return out
